# revision 1
# baseline (speedup 1.0000x reference)
"""Trainium2 Bass kernel for nn_MultiHeadAttention (B=4, S=2048, D=1024, H=16).

Sharding: tensor-parallel over heads (2 heads per core, 8 cores). Each core:
  1. Projects Q/K (feature-major, [128 feats x 8192 seq]) and V (seq-major via
     PE transpose, augmented with a ones-column for the softmax denominator).
  2. Computes causal attention for its 8 (batch, head) pairs in bf16 with
     fp32 PSUM accumulation: scoresT = K-chunk @ Q-strip, exp on ACT,
     AV+denominator via one accumulating matmul against [V | 1].
  3. A per-batch AllGather (bf16 payload) publishes attention outputs; the
     first three overlap the next batch's compute. Each core then pulls the
     features of its own 1024-row sequence block via an indirect (index-
     driven) DMA gather -- the indices come from a per-core input tensor, so
     the SPMD program stays identical across cores -- and computes its block
     of the output projection in bf16. A chained dummy-matmul "warm keeper"
     spans the final AllGather wait to hold the PE clock at 2.4 GHz.
Host wraps: shards weights (with 1/sqrt(dk) folded into Wq), classifies mask
blocks (skip / keep / masked via unique [128, 512] tiles), and reassembles
the full [4, 2048, 1024] output.
"""

import ml_dtypes
import numpy as np

import concourse.bass as bass
import concourse.bacc as bacc
import concourse.mybir as mybir
import concourse.tile as tile
from concourse.bass_utils import run_bass_kernel_spmd

F32 = mybir.dt.float32
F32R = mybir.dt.float32r
BF16 = mybir.dt.bfloat16
AF = mybir.ActivationFunctionType
OP = mybir.AluOpType

B, S, D_MODEL, N_HEADS, D_K = 4, 2048, 1024, 16, 64
N_CORES = 8
HPC = N_HEADS // N_CORES          # heads per core = 2
F = HPC * D_K                     # feature slice per core = 128
SEQ = B * S                       # 8192
S1B = 512                         # query-strip width (scores free dim)
S2B = 128                         # key-block height (scores partition dim)
SP = S // S1B                     # 4 strips per batch
C2 = S // S2B                     # 16 key chunks per batch
KC = D_MODEL // 128               # 8 contraction chunks for projections
SC_GLOBAL = SEQ // S1B            # 16 projection seq strips
A_DROP, A_KEEP = -2, -1

_nc_cache = {}


def _build_nc(actions_key, n_masks):
    actions = np.frombuffer(actions_key, dtype=np.int64).reshape(C2, SP)
    nc = bacc.Bacc("TRN2", target_bir_lowering=False, debug=False,
                   num_devices=N_CORES)

    xq = nc.dram_tensor("xq", [D_MODEL, SEQ], BF16, kind="ExternalInput")
    xk = nc.dram_tensor("xk", [D_MODEL, SEQ], BF16, kind="ExternalInput")
    xv = nc.dram_tensor("xv", [D_MODEL, SEQ], BF16, kind="ExternalInput")
    wq = nc.dram_tensor("wq", [D_MODEL, F], BF16, kind="ExternalInput")
    wk = nc.dram_tensor("wk", [D_MODEL, F], BF16, kind="ExternalInput")
    wv = nc.dram_tensor("wv", [D_MODEL, F], BF16, kind="ExternalInput")
    bq = nc.dram_tensor("bq", [F, 1], F32, kind="ExternalInput")
    bk = nc.dram_tensor("bk", [F, 1], F32, kind="ExternalInput")
    bv = nc.dram_tensor("bv", [F, 1], F32, kind="ExternalInput")
    woT = nc.dram_tensor("woT", [D_MODEL, D_MODEL], BF16, kind="ExternalInput")
    bo = nc.dram_tensor("bo", [KC, 128, 1], F32, kind="ExternalInput")
    ident = nc.dram_tensor("ident", [128, 128], BF16, kind="ExternalInput")
    masks = nc.dram_tensor("masks", [max(n_masks, 1), S2B, S1B], BF16,
                           kind="ExternalInput")

    oidx = nc.dram_tensor("oidx", [KC, 128, 1], mybir.dt.int32, kind="ExternalInput")
    agin = nc.dram_tensor("agin", [B, F, S], BF16)
    agf = nc.dram_tensor("agf", [B, N_CORES, F, S], BF16, addr_space="Shared")
    out_t = nc.dram_tensor("out_t", [D_MODEL, SEQ // N_CORES], F32,
                           kind="ExternalOutput")

    with tile.TileContext(nc) as tc:
      with tc.tile_pool(name="oproj_w", bufs=1) as opw:
        wo_sb = opw.tile([128, KC, KC, 128], BF16, tag="wo")
        bo_sb = opw.tile([128, KC], F32, tag="bo")
        with (
            tc.tile_pool(name="const", bufs=1) as cst,
            tc.tile_pool(name="persist", bufs=1) as per,
            tc.tile_pool(name="xin", bufs=16) as xin,
            tc.tile_pool(name="vtmp", bufs=2) as vtmp,
            tc.tile_pool(name="probs", bufs=8) as prp,
            tc.tile_pool(name="norm", bufs=3) as nrm,
            tc.tile_pool(name="pp_ps", bufs=2, space="PSUM") as pp_ps,
            tc.tile_pool(name="sc_ps", bufs=2, space="PSUM") as sc_ps,
            tc.tile_pool(name="av_ps", bufs=2, space="PSUM") as av_ps,
        ):
            wq_sb = cst.tile([128, KC, F], BF16, tag="wq")
            wk_sb = cst.tile([128, KC, F], BF16, tag="wk")
            wv_sb = cst.tile([128, KC, F], BF16, tag="wv")
            nc.sync.dma_start(wq_sb[:], wq[:].rearrange("(kc p) f -> p kc f", p=128))
            nc.sync.dma_start(wk_sb[:], wk[:].rearrange("(kc p) f -> p kc f", p=128))
            nc.sync.dma_start(wv_sb[:], wv[:].rearrange("(kc p) f -> p kc f", p=128))
            bq_sb = cst.tile([F, 1], F32, tag="bq")
            bk_sb = cst.tile([F, 1], F32, tag="bk")
            bv_sb = cst.tile([F, 1], F32, tag="bv")
            nc.sync.dma_start(bq_sb[:], bq[:])
            nc.sync.dma_start(bk_sb[:], bk[:])
            nc.sync.dma_start(bv_sb[:], bv[:])
            id_sb = cst.tile([128, 128], BF16, tag="id")
            nc.sync.dma_start(id_sb[:], ident[:])
            mk_sb = cst.tile([S2B, max(n_masks, 1), S1B], BF16, tag="mk")
            nc.sync.dma_start(mk_sb[:], masks[:].rearrange("n p f -> p n f"))

            qT = per.tile([F, SEQ], BF16, tag="qT")
            kT = per.tile([F, SEQ], BF16, tag="kT")
            # V (seq-major) with per-head ones column: [s2_in_chunk, b, c2, h, dk+1]
            v_aug = per.tile([S2B, B, C2, HPC, D_K + 1], BF16, tag="vaug")
            ones_sb = cst.tile([128, 1], F32, tag="ones")
            nc.vector.memset(ones_sb[:], 1.0)
            ones_r = cst.tile([1, D_K], F32R, tag="onesr")
            nc.vector.tensor_copy(ones_r[:], ones_sb[0:1, 0:1].to_broadcast([1, D_K]))

            for b in range(B):
                # --- projections for batch b (per tensor: load 8 k-chunks
                # of the batch as [128, 2048] tiles, project 4 strips) ---
                for name, x_dram, w_sb, b_sb in (
                    ("q", xq, wq_sb, bq_sb),
                    ("k", xk, wk_sb, bk_sb),
                    ("v", xv, wv_sb, bv_sb),
                ):
                    xts = []
                    for kc in range(KC):
                        xt = xin.tile([128, S], BF16, tag="xt")
                        nc.sync.dma_start(
                            xt[:], x_dram[kc * 128:(kc + 1) * 128,
                                          b * S:(b + 1) * S])
                        xts.append(xt)
                    for sc_local in range(SP):
                        sl = slice(sc_local * S1B, (sc_local + 1) * S1B)
                        gsl = slice(b * S + sc_local * S1B,
                                    b * S + (sc_local + 1) * S1B)
                        ps = pp_ps.tile([128, S1B], F32, tag="pp")
                        for kc in range(KC):
                            nc.tensor.matmul(ps[:], w_sb[:, kc, :], xts[kc][:, sl],
                                             start=(kc == 0), stop=(kc == KC - 1))
                        if name == "q":
                            nc.vector.tensor_scalar_add(qT[:, gsl], ps[:], b_sb[:, 0:1])
                        elif name == "k":
                            nc.vector.tensor_scalar_add(kT[:, gsl], ps[:], b_sb[:, 0:1])
                        else:
                            vt = vtmp.tile([128, S1B], BF16, tag="vt")
                            nc.vector.tensor_scalar_add(vt[:], ps[:], b_sb[:, 0:1])
                            for j in range(S1B // 128):
                                tp = pp_ps.tile([128, 128], BF16, tag="pp")
                                nc.tensor.transpose(tp[:], vt[:, j * 128:(j + 1) * 128],
                                                    id_sb[:])
                                c2 = sc_local * (S1B // 128) + j
                                nc.vector.tensor_copy(
                                    v_aug[:, b, c2, :, 0:D_K],
                                    tp[:].rearrange("p (h d) -> p h d", h=HPC))
                                nc.vector.tensor_copy(
                                    v_aug[:, b, c2, :, D_K:D_K + 1],
                                    ones_sb[:, :, None].to_broadcast([S2B, HPC, 1]))

                # --- attention for batch b, both local heads interleaved ---
                for i1 in range(SP):
                    kept = [i2 for i2 in range(C2) if actions[i2, i1] != A_DROP]
                    avs = []
                    for _lh in range(HPC):
                        av_t = av_ps.tile([D_K + 1, S1B], F32, tag="av")
                        avs.append(av_t)
                    n_done = 0
                    pend = None  # (g, prs_all, start_idx) awaiting AV emission
                    def emit_av(pend_g, pend_prs, start_idx):
                        for lh in range(HPC):
                            for idx, i2 in enumerate(pend_g):
                                a = actions[i2, i1]
                                prs = pend_prs[lh][:, idx * S1B:(idx + 1) * S1B]
                                if a >= 0:
                                    nc.vector.tensor_tensor(
                                        prs, prs, mk_sb[:, a, :], OP.mult)
                                nc.tensor.matmul(
                                    avs[lh][:], v_aug[:, b, i2, lh, :], prs,
                                    start=(start_idx + idx == 0),
                                    stop=(start_idx + idx == len(kept) - 1))
                    while n_done < len(kept):
                        g = kept[n_done:n_done + 2]
                        prs_all = []
                        for lh in range(HPC):
                            r0, r1 = lh * D_K, (lh + 1) * D_K
                            q_strip = qT[r0:r1,
                                         b * S + i1 * S1B: b * S + (i1 + 1) * S1B]
                            sc_t = sc_ps.tile([128, S1B * 2], F32, tag="sc")
                            for idx, i2 in enumerate(g):
                                nc.tensor.matmul(
                                    sc_t[:, idx * S1B:(idx + 1) * S1B],
                                    kT[r0:r1, b * S + i2 * S2B: b * S + i2 * S2B + S2B],
                                    q_strip, start=True, stop=True)
                            pr = prp.tile([128, S1B * 2], BF16, tag="pr")
                            nc.scalar.activation(pr[:, 0:len(g) * S1B],
                                                 sc_t[:, 0:len(g) * S1B], AF.Exp)
                            prs_all.append(pr)
                        if pend is not None:
                            emit_av(*pend)
                        pend = (g, prs_all, n_done)
                        n_done += len(g)
                    if pend is not None:
                        emit_av(*pend)

                    # copy out of PSUM immediately to release the banks
                    for lh in range(HPC):
                        r0, r1 = lh * D_K, (lh + 1) * D_K
                        avc = nrm.tile([D_K + 1, S1B], F32, tag="avc")
                        nc.vector.tensor_copy(avc[:], avs[lh][:])
                        rcp = nrm.tile([1, S1B], F32R, tag="rcp")
                        with nc.allow_low_precision("f32r recip feeds bcast matmul"):
                            nc.vector.reciprocal(rcp[:], avc[D_K:D_K + 1, :])
                        bc_ps = av_ps.tile([D_K, S1B], F32, tag="av")
                        nc.tensor.matmul(bc_ps[:], ones_r[:], rcp[:],
                                         start=True, stop=True)
                        ob = nrm.tile([D_K, S1B], BF16, tag="ob")
                        nc.vector.tensor_tensor(ob[:], avc[0:D_K, :], bc_ps[:],
                                                OP.mult)
                        nc.sync.dma_start(
                            agin[b, r0:r1, i1 * S1B:(i1 + 1) * S1B], ob[:])

                nc.gpsimd.collective_compute(
                    "AllGather", OP.bypass,
                    ins=[agin[b]], outs=[agf[b]],
                    replica_groups=[list(range(N_CORES))])



        nc.sync.dma_start(
            wo_sb[:],
            woT[:].rearrange("(kc p) (dc f) -> p kc dc f", p=128, f=128))
        nc.sync.dma_start(bo_sb[:], bo[:].rearrange("d p one -> p (d one)"))
        with (
            tc.tile_pool(name="oproj", bufs=1) as opr,
            tc.tile_pool(name="ob_sb", bufs=3) as obp,
            tc.tile_pool(name="op_ps", bufs=2, space="PSUM") as op_ps,
        ):
            # warm-keeper: chained dummy matmuls span the AllGather wait so
            # the PE clock stays at 2.4GHz for the output projection
            wsb = opr.tile([128, S1B], BF16, tag="wsb")
            nc.vector.tensor_copy(wsb[:], wo_sb[:, 0, 0:4, :].rearrange("p a f -> p (a f)"))
            for _ in range(22):
                wps = op_ps.tile([128, S1B], F32, tag="op")
                nc.tensor.matmul(wps[:], wo_sb[:, 0, 0, :], wsb[:],
                                 start=True, stop=True)
                wsb = opr.tile([128, S1B], BF16, tag="wsb")
                nc.vector.tensor_copy(wsb[:], wps[:])
            idx_sb = opr.tile([128, KC], mybir.dt.int32, tag="idx")
            nc.sync.dma_start(idx_sb[:], oidx[:].rearrange("g p one -> p (g one)"))
            agf_rows = agf[:].rearrange("b g p (h s) -> (b g p h) s", h=2)
            rhs = opr.tile([128, KC, SEQ // N_CORES], BF16, tag="rhs")
            for g in range(KC):
                nc.gpsimd.indirect_dma_start(
                    out=rhs[:, g, :], out_offset=None,
                    in_=agf_rows,
                    in_offset=bass.IndirectOffsetOnAxis(ap=idx_sb[:, g:g + 1], axis=0))
            n_sc2 = (SEQ // N_CORES) // S1B
            for dc in range(KC):
                for sc2 in range(n_sc2):
                    ps = op_ps.tile([128, S1B], F32, tag="op")
                    for kc in range(KC):
                        nc.tensor.matmul(
                            ps[:], wo_sb[:, kc, dc, :],
                            rhs[:, kc, sc2 * S1B:(sc2 + 1) * S1B],
                            start=(kc == 0), stop=(kc == KC - 1))
                    ob = obp.tile([128, S1B], F32, tag="obt")
                    nc.vector.tensor_scalar_add(ob[:], ps[:], bo_sb[:, dc:dc + 1])
                    nc.sync.dma_start(
                        out_t[dc * 128:(dc + 1) * 128,
                              sc2 * S1B:(sc2 + 1) * S1B], ob[:])

    nc.finalize()
    return nc


def _classify_mask(mask):
    """Block-classify mask[0,0] on the scoresT grid: per (key-chunk i2,
    query-strip i1) -> drop / keep / index of a unique [128, 512] 0/1 tile."""
    m2 = np.asarray(mask)[0, 0] != 0  # [S, S], m2[q, k]
    actions = np.full((C2, SP), A_DROP, dtype=np.int64)
    uniq, tiles = {}, []
    for i2 in range(C2):
        for i1 in range(SP):
            blk = m2[i1 * S1B:(i1 + 1) * S1B, i2 * S2B:(i2 + 1) * S2B].T
            if blk.all():
                actions[i2, i1] = A_KEEP
            elif blk.any():
                key = blk.tobytes()
                if key not in uniq:
                    uniq[key] = len(tiles)
                    tiles.append(np.ascontiguousarray(blk).astype(ml_dtypes.bfloat16))
                actions[i2, i1] = uniq[key]
    arr = (np.stack(tiles) if tiles
           else np.zeros((1, S2B, S1B), dtype=ml_dtypes.bfloat16))
    return actions, arr


def _prep(inputs):
    q = np.asarray(inputs["query"], dtype=np.float32).reshape(SEQ, D_MODEL)
    k = np.asarray(inputs["key"], dtype=np.float32).reshape(SEQ, D_MODEL)
    v = np.asarray(inputs["value"], dtype=np.float32).reshape(SEQ, D_MODEL)
    bf = ml_dtypes.bfloat16
    xq = np.ascontiguousarray(q.T).astype(bf)
    xk = np.ascontiguousarray(k.T).astype(bf)
    xv = np.ascontiguousarray(v.T).astype(bf)

    Wq = np.asarray(inputs["Wq"], dtype=np.float32)
    Wk = np.asarray(inputs["Wk"], dtype=np.float32)
    Wv = np.asarray(inputs["Wv"], dtype=np.float32)
    Wo = np.asarray(inputs["Wo"], dtype=np.float32)
    bq = np.asarray(inputs["bq"], dtype=np.float32)
    bk = np.asarray(inputs["bk"], dtype=np.float32)
    bv = np.asarray(inputs["bv"], dtype=np.float32)
    bo = np.asarray(inputs["bo"], dtype=np.float32)

    scale = 1.0 / np.sqrt(D_K)
    actions, mask_tiles = _classify_mask(inputs["mask"])

    # exp-overflow guard for the no-max-subtract softmax (Cauchy-Schwarz bound)
    qn = q @ Wq.T + bq
    kn = k @ Wk.T + bk
    qmax = np.linalg.norm(qn.reshape(SEQ, N_HEADS, D_K), axis=-1).max()
    kmax = np.linalg.norm(kn.reshape(SEQ, N_HEADS, D_K), axis=-1).max()
    assert scale * qmax * kmax < 80.0, "score bound too large for exp without max-subtraction"

    shared = {
        "xq": xq, "xk": xk, "xv": xv,
        "woT": np.ascontiguousarray(Wo.T).astype(bf),
        "bo": np.ascontiguousarray(bo.reshape(KC, 128, 1)),
        "ident": np.eye(128, dtype=np.float32).astype(bf),
        "masks": mask_tiles,
    }
    in_maps = []
    for c in range(N_CORES):
        sl = slice(c * F, (c + 1) * F)
        m = dict(shared)
        m["wq"] = np.ascontiguousarray((Wq[sl] * scale).T).astype(bf)
        m["wk"] = np.ascontiguousarray(Wk[sl].T).astype(bf)
        m["wv"] = np.ascontiguousarray(Wv[sl].T).astype(bf)
        m["bq"] = np.ascontiguousarray((bq[sl] * scale).reshape(F, 1))
        m["bk"] = np.ascontiguousarray(bk[sl].reshape(F, 1))
        m["bv"] = np.ascontiguousarray(bv[sl].reshape(F, 1))
        gg, pp = np.meshgrid(np.arange(KC), np.arange(128), indexing="ij")
        m["oidx"] = np.ascontiguousarray(
            (((((c // 2) * KC + gg) * 128 + pp) * 2) + (c % 2))
            .reshape(KC, 128, 1).astype(np.int32))
        in_maps.append(m)
    return in_maps, actions, mask_tiles


def _run(inputs, trace=False, trace_cores=None):
    in_maps, actions, mask_tiles = _prep(inputs)
    key = (actions.tobytes(), len(mask_tiles))
    if key not in _nc_cache:
        _nc_cache[key] = _build_nc(key[0], key[1])
    nc = _nc_cache[key]
    res = run_bass_kernel_spmd(nc, in_maps, list(range(N_CORES)),
                               trace=trace, trace_cores=trace_cores)
    blk = SEQ // N_CORES
    out = np.empty((SEQ, D_MODEL), dtype=np.float32)
    for c in range(N_CORES):
        out[c * blk:(c + 1) * blk] = res.results[c]["out_t"].T
    return out.reshape(B, S, D_MODEL), res


def kernel(**inputs) -> np.ndarray:
    out, _ = _run(inputs)
    return out



# revision 27
# speedup vs baseline: 1.1167x; 1.1167x over previous
"""Trainium2 Bass kernel for nn_MultiHeadAttention (B=4, S=2048, D=1024, H=16).

Sharding: tensor-parallel over heads (2 heads per core, 8 cores). Each core:
  1. Projects Q/K (feature-major, [128 feats x 8192 seq]) and V (seq-major via
     PE transpose).
  2. Computes causal attention for its 8 (batch, head) pairs in bf16 with
     fp32 PSUM accumulation. PE-array packing: the two heads' score matmuls
     (K=64 contraction) run concurrently in disjoint row-groups; the two
     heads' AV matmuls (M=64) run concurrently in disjoint col-groups; the
     softmax denominators are M=1 matmuls against a ones column, packed into
     col-tiles 0/32. Projection matmuls for batch b+1 are interleaved into
     the attention instruction stream of batch b to keep the PE dense (HAM
     stays un-throttled).
  3. Normalization: reciprocal_approx_fast on the two denominator rows, one
     broadcast matmul expands them across both heads' 64 features, one DVE
     multiply, then a per-strip AllGather (bf16) publishes the [128, 512]
     block. 16 small per-strip gathers replace 4 big per-batch ones so the
     final gather tail is short. Each core then pulls the features of its own
     1024-row sequence block via indirect (index-driven) DMA -- indices come
     from a per-core input tensor so the SPMD program is identical across
     cores -- and computes its block of the output projection. A chained
     dummy-matmul warm keeper spans the final gather wait.
Host wraps: shards weights (with 1/sqrt(dk) folded into Wq), classifies mask
blocks (skip / keep / masked via unique [128, 512] tiles), and reassembles
the full [4, 2048, 1024] output.
"""

import ml_dtypes
import numpy as np

import concourse.bass as bass
import concourse.bacc as bacc
import concourse.mybir as mybir
import concourse.tile as tile
from concourse.bass_utils import run_bass_kernel_spmd

F32 = mybir.dt.float32
F32R = mybir.dt.float32r
BF16 = mybir.dt.bfloat16
AF = mybir.ActivationFunctionType
OP = mybir.AluOpType

B, S, D_MODEL, N_HEADS, D_K = 4, 2048, 1024, 16, 64
N_CORES = 8
HPC = N_HEADS // N_CORES          # heads per core = 2
F = HPC * D_K                     # feature slice per core = 128
SEQ = B * S                       # 8192
S1B = 512                         # query-strip width (scores free dim)
S2B = 128                         # key-block height (scores partition dim)
SP = S // S1B                     # 4 strips per batch
C2 = S // S2B                     # 16 key chunks per batch
KC = D_MODEL // 128               # 8 contraction chunks for projections
A_DROP, A_KEEP = -2, -1

_nc_cache = {}


def _build_nc(actions_key, n_masks, debug=False):
    actions = np.frombuffer(actions_key, dtype=np.int64).reshape(C2, SP)
    nc = bacc.Bacc("TRN2", target_bir_lowering=False, debug=False,
                   num_devices=N_CORES)

    xq = nc.dram_tensor("xq", [D_MODEL, SEQ], BF16, kind="ExternalInput")
    xk = nc.dram_tensor("xk", [D_MODEL, SEQ], BF16, kind="ExternalInput")
    xv = nc.dram_tensor("xv", [D_MODEL, SEQ], BF16, kind="ExternalInput")
    wq = nc.dram_tensor("wq", [D_MODEL, F], BF16, kind="ExternalInput")
    wk = nc.dram_tensor("wk", [D_MODEL, F], BF16, kind="ExternalInput")
    wv = nc.dram_tensor("wv", [D_MODEL, F], BF16, kind="ExternalInput")
    bq = nc.dram_tensor("bq", [F, 1], F32, kind="ExternalInput")
    bk = nc.dram_tensor("bk", [F, 1], F32, kind="ExternalInput")
    bv = nc.dram_tensor("bv", [F, 1], F32, kind="ExternalInput")
    woT = nc.dram_tensor("woT", [D_MODEL, D_MODEL], BF16, kind="ExternalInput")
    bo = nc.dram_tensor("bo", [KC, 128, 1], F32, kind="ExternalInput")
    ident = nc.dram_tensor("ident", [128, 128], BF16, kind="ExternalInput")
    masks = nc.dram_tensor("masks", [max(n_masks, 1), S2B, S1B], BF16,
                           kind="ExternalInput")

    oidx = nc.dram_tensor("oidx", [128, KC * 2], mybir.dt.int32,
                          kind="ExternalInput")
    agin = nc.dram_tensor("agin", [B, SP, F, S1B], BF16)
    agf = nc.dram_tensor("agf", [B, SP, N_CORES, F, S1B], BF16,
                         addr_space="Shared")
    out_t = nc.dram_tensor("out_t", [D_MODEL, SEQ // N_CORES], F32,
                           kind="ExternalOutput")
    if debug:
        dbg_ob = nc.dram_tensor("dbg_ob", [B, SP, F, S1B], BF16,
                                kind="ExternalOutput")
        dbg_avc = nc.dram_tensor("dbg_avc", [B, SP, F, S1B], F32,
                                 kind="ExternalOutput")
        dbg_rcp = nc.dram_tensor("dbg_rcp", [B, SP, 64, S1B], F32,
                                 kind="ExternalOutput")
        dbg_rhs = nc.dram_tensor("dbg_rhs", [128, KC, 2, S1B], BF16,
                                 kind="ExternalOutput")

    with tile.TileContext(nc) as tc:
      with tc.tile_pool(name="oproj_w", bufs=1) as opw:
        wo_sb = opw.tile([128, KC, KC, 128], BF16, tag="wo")
        bo_sb = opw.tile([128, KC], F32, tag="bo")
        with (
            tc.tile_pool(name="const", bufs=1) as cst,
            tc.tile_pool(name="persist", bufs=1) as per,
            tc.tile_pool(name="xin", bufs=16) as xin,
            tc.tile_pool(name="vtmp", bufs=2) as vtmp,
            tc.tile_pool(name="probs", bufs=8) as prp,
            tc.tile_pool(name="norm", bufs=4) as nrm,
            tc.tile_pool(name="obuf", bufs=3) as obp,
            tc.tile_pool(name="pp_ps", bufs=2, space="PSUM") as pp_ps,
            tc.tile_pool(name="sc_ps", bufs=2, space="PSUM") as sc_ps,
            tc.tile_pool(name="av_ps", bufs=2, space="PSUM") as av_ps,
        ):
            wq_sb = cst.tile([128, KC, F], BF16, tag="wq")
            wk_sb = cst.tile([128, KC, F], BF16, tag="wk")
            wv_sb = cst.tile([128, KC, F], BF16, tag="wv")
            nc.sync.dma_start(wq_sb[:], wq[:].rearrange("(kc p) f -> p kc f", p=128))
            nc.sync.dma_start(wk_sb[:], wk[:].rearrange("(kc p) f -> p kc f", p=128))
            nc.sync.dma_start(wv_sb[:], wv[:].rearrange("(kc p) f -> p kc f", p=128))
            bq_sb = cst.tile([F, 1], F32, tag="bq")
            bk_sb = cst.tile([F, 1], F32, tag="bk")
            bv_sb = cst.tile([F, 1], F32, tag="bv")
            nc.sync.dma_start(bq_sb[:], bq[:])
            nc.sync.dma_start(bk_sb[:], bk[:])
            nc.sync.dma_start(bv_sb[:], bv[:])
            id_sb = cst.tile([128, 128], BF16, tag="id")
            nc.sync.dma_start(id_sb[:], ident[:])
            mk_sb = cst.tile([S2B, max(n_masks, 1), S1B], BF16, tag="mk")
            nc.sync.dma_start(mk_sb[:], masks[:].rearrange("n p f -> p n f"))

            qT = per.tile([F, SEQ], BF16, tag="qT")
            kT = per.tile([F, SEQ], BF16, tag="kT")
            # V (seq-major) with per-head ones column for the softmax
            # denominator: [s2_in_chunk, b, c2, h, dk+1]
            v_aug = per.tile([S2B, B, C2, HPC, D_K + 1], BF16, tag="vaug")
            ones_sb = cst.tile([128, 1], F32, tag="ones")
            nc.vector.memset(ones_sb[:], 1.0)
            ones_r = cst.tile([1, D_K], F32, tag="onesr")
            nc.vector.tensor_copy(ones_r[:], ones_sb[0:1, 0:1].to_broadcast([1, D_K]))

            def make_proj_units(b):
                """Emission closures for batch b's projections; popped as
                PE filler between attention groups of batch b-1."""
                units = []
                xts_map = {}

                def dma_unit():
                    for name, x_dram in (("q", xq), ("k", xk), ("v", xv)):
                        xts = []
                        for kc in range(KC):
                            xt = xin.tile([128, S], BF16, tag="xt")
                            nc.sync.dma_start(
                                xt[:], x_dram[kc * 128:(kc + 1) * 128,
                                              b * S:(b + 1) * S])
                            xts.append(xt)
                        xts_map[name] = xts
                units.append(dma_unit)

                def proj_unit(name, w_sb, b_sb, sc_local):
                    xts = xts_map[name]
                    sl = slice(sc_local * S1B, (sc_local + 1) * S1B)
                    gsl = slice(b * S + sc_local * S1B,
                                b * S + (sc_local + 1) * S1B)
                    ps = pp_ps.tile([128, S1B], F32, tag="pp")
                    for kc in range(KC):
                        nc.tensor.matmul(ps[:], w_sb[:, kc, :], xts[kc][:, sl],
                                         start=(kc == 0), stop=(kc == KC - 1))
                    if name == "q":
                        nc.vector.tensor_scalar_add(qT[:, gsl], ps[:], b_sb[:, 0:1])
                    elif name == "k":
                        nc.vector.tensor_scalar_add(kT[:, gsl], ps[:], b_sb[:, 0:1])
                    else:
                        vt = vtmp.tile([128, S1B], BF16, tag="vt")
                        nc.vector.tensor_scalar_add(vt[:], ps[:], b_sb[:, 0:1])
                        for j in range(S1B // 128):
                            tp = pp_ps.tile([128, 128], BF16, tag="pp")
                            nc.tensor.transpose(tp[:], vt[:, j * 128:(j + 1) * 128],
                                                id_sb[:])
                            c2 = sc_local * (S1B // 128) + j
                            nc.vector.tensor_copy(
                                v_aug[:, b, c2, :, 0:D_K],
                                tp[:].rearrange("p (h d) -> p h d", h=HPC))
                            nc.vector.tensor_copy(
                                v_aug[:, b, c2, :, D_K:D_K + 1],
                                ones_sb[:, :, None].to_broadcast([S2B, HPC, 1]))

                for name, w_sb, b_sb in (("q", wq_sb, bq_sb),
                                         ("k", wk_sb, bk_sb),
                                         ("v", wv_sb, bv_sb)):
                    for sc_local in range(SP):
                        units.append(
                            lambda n=name, w=w_sb, bb=b_sb, s=sc_local:
                                proj_unit(n, w, bb, s))
                return units

            def emit_attention(b, fillers):
                """Attention for batch b; pops filler closures (batch b+1
                projections) between chunk groups to keep the PE dense."""
                # fillers[0] is the DMA unit: emit immediately for max lead.
                fidx = 0
                if fillers:
                    fillers[0]()
                    fidx = 1
                n_groups = sum(
                    len([i2 for i2 in range(C2) if actions[i2, i1] != A_DROP])
                    for i1 in range(SP)) // 2
                gcount = 0
                for i1 in range(SP):
                    kept = [i2 for i2 in range(C2) if actions[i2, i1] != A_DROP]
                    avs = [av_ps.tile([D_K + 1, S1B], F32, tag="av", name="av0"),
                           av_ps.tile([D_K + 1, S1B], F32, tag="av", name="av1")]
                    q_lo = b * S + i1 * S1B

                    def emit_av(pend_g, pend_prs, start_idx, kept=kept,
                                avs=avs):
                        n_k = len(kept)
                        for idx, i2 in enumerate(pend_g):
                            first = (start_idx + idx == 0)
                            last = (start_idx + idx == n_k - 1)
                            for lh in range(HPC):
                                nc.tensor.matmul(
                                    avs[lh][:],
                                    v_aug[:, b, i2, lh, :],
                                    pend_prs[lh][:, idx * S1B:(idx + 1) * S1B],
                                    start=first, stop=last)

                    n_done = 0
                    pend = None
                    while n_done < len(kept):
                        g = kept[n_done:n_done + 2]
                        sc_ts, prs = [], []
                        for lh in range(HPC):
                            sc_ts.append(sc_ps.tile([128, S1B * 2], F32,
                                                    tag="sc", name="sc_t"))
                            prs.append(prp.tile([128, S1B * 2], BF16,
                                                tag="pr", name="pr"))
                        # interleave heads per chunk: disjoint row-groups
                        # (h0: partitions 0-63, h1: 64-127) run concurrently
                        for idx, i2 in enumerate(g):
                            k_lo = b * S + i2 * S2B
                            for lh in range(HPC):
                                r0 = lh * D_K
                                nc.tensor.matmul(
                                    sc_ts[lh][:, idx * S1B:(idx + 1) * S1B],
                                    kT[r0:r0 + D_K, k_lo:k_lo + S2B],
                                    qT[r0:r0 + D_K, q_lo:q_lo + S1B],
                                    start=True, stop=True)
                        for lh in range(HPC):
                            nc.scalar.activation(prs[lh][:, 0:len(g) * S1B],
                                                 sc_ts[lh][:, 0:len(g) * S1B],
                                                 AF.Exp)
                        for lh in range(HPC):
                            for idx, i2 in enumerate(g):
                                a = actions[i2, i1]
                                if a >= 0:
                                    pr_sl = prs[lh][:, idx * S1B:(idx + 1) * S1B]
                                    nc.vector.tensor_tensor(
                                        pr_sl, pr_sl, mk_sb[:, a, :], OP.mult)
                        if pend is not None:
                            emit_av(*pend)
                        pend = (g, prs, n_done)
                        n_done += len(g)
                        gcount += 1
                        while (fidx < len(fillers)
                               and fidx - 1 < (gcount * (len(fillers) - 1)
                                               ) // n_groups):
                            fillers[fidx]()
                            fidx += 1
                    if pend is not None:
                        emit_av(*pend)

                    # normalization per head: copy out of PSUM, fast
                    # reciprocal of the denominator row, broadcast matmul
                    for lh in range(HPC):
                        r0, r1 = lh * D_K, (lh + 1) * D_K
                        avc = nrm.tile([D_K, S1B], F32, tag="avc")
                        nc.vector.tensor_copy(avc[:], avs[lh][0:D_K, :])
                        # shift the denominator row to partition 0 (regular
                        # DVE copy supports base shift; the custom-DVE
                        # reciprocal does not)
                        den = nrm.tile([1, S1B], F32, tag="den")
                        nc.vector.tensor_copy(den[:], avs[lh][D_K:D_K + 1, :])
                        rcp = nrm.tile([1, S1B], F32, tag="rcp")
                        nc.vector.reciprocal_approx_fast(rcp[:], den[:])
                        bc_ps = pp_ps.tile([D_K, S1B], F32, tag="pp")
                        nc.tensor.matmul(bc_ps[:], ones_r[:], rcp[:],
                                         start=True, stop=True)
                        ob = obp.tile([D_K, S1B], BF16, tag="ob")
                        nc.vector.tensor_tensor(ob[:], avc[:], bc_ps[:],
                                                OP.mult)
                        nc.sync.dma_start(agin[b, i1, r0:r1, :], ob[:])
                        if debug:
                            nc.sync.dma_start(dbg_ob[b, i1, r0:r1, :], ob[:])
                            nc.sync.dma_start(dbg_avc[b, i1, r0:r1, :], avc[:])
                            nc.sync.dma_start(
                                dbg_rcp[b, i1, lh * 32:lh * 32 + 1, :],
                                rcp[:])
                    nc.gpsimd.collective_compute(
                        "AllGather", OP.bypass,
                        ins=[agin[b, i1]], outs=[agf[b, i1]],
                        replica_groups=[list(range(N_CORES))])
                while fidx < len(fillers):
                    fillers[fidx]()
                    fidx += 1

            # batch 0 projections up-front, then attention(b) interleaved
            # with projections(b+1)
            units0 = make_proj_units(0)
            for u in units0:
                u()
            for b in range(B):
                fillers = make_proj_units(b + 1) if b + 1 < B else []
                emit_attention(b, fillers)

        nc.sync.dma_start(
            wo_sb[:],
            woT[:].rearrange("(kc p) (dc f) -> p kc dc f", p=128, f=128))
        nc.sync.dma_start(bo_sb[:], bo[:].rearrange("d p one -> p (d one)"))
        with (
            tc.tile_pool(name="oproj", bufs=1) as opr,
            tc.tile_pool(name="ob_sb", bufs=3) as obp2,
            tc.tile_pool(name="op_ps", bufs=2, space="PSUM") as op_ps,
        ):
            # warm-keeper: chained dummy matmuls span the final gather wait
            # so the PE clock stays at 2.4GHz for the output projection
            wsb = opr.tile([128, S1B], BF16, tag="wsb")
            nc.vector.tensor_copy(wsb[:], wo_sb[:, 0, 0:4, :].rearrange("p a f -> p (a f)"))
            for _ in range(16):
                wps = op_ps.tile([128, S1B], F32, tag="op")
                nc.tensor.matmul(wps[:], wo_sb[:, 0, 0, :], wsb[:],
                                 start=True, stop=True)
                wsb = opr.tile([128, S1B], BF16, tag="wsb")
                nc.vector.tensor_copy(wsb[:], wps[:])
            idx_sb = opr.tile([128, KC * 2], mybir.dt.int32, tag="idx")
            nc.sync.dma_start(idx_sb[:], oidx[:])
            agf_rows = agf[:].rearrange("b s g p f -> (b s g p) f")
            rhs = opr.tile([128, KC, 2, S1B], BF16, tag="rhs")
            for s in range(2):
                for g in range(KC):
                    nc.gpsimd.indirect_dma_start(
                        out=rhs[:, g, s, :], out_offset=None,
                        in_=agf_rows,
                        in_offset=bass.IndirectOffsetOnAxis(
                            ap=idx_sb[:, g * 2 + s:g * 2 + s + 1], axis=0))
            if debug:
                nc.sync.dma_start(dbg_rhs[:], rhs[:])
            for dc in range(KC):
                for sc2 in range(2):
                    ps = op_ps.tile([128, S1B], F32, tag="op")
                    for kc in range(KC):
                        nc.tensor.matmul(
                            ps[:], wo_sb[:, kc, dc, :],
                            rhs[:, kc, sc2, :],
                            start=(kc == 0), stop=(kc == KC - 1))
                    ob = obp2.tile([128, S1B], F32, tag="obt")
                    nc.vector.tensor_scalar_add(ob[:], ps[:], bo_sb[:, dc:dc + 1])
                    nc.sync.dma_start(
                        out_t[dc * 128:(dc + 1) * 128,
                              sc2 * S1B:(sc2 + 1) * S1B], ob[:])

    nc.finalize()
    return nc


def _classify_mask(mask):
    """Block-classify mask[0,0] on the scoresT grid: per (key-chunk i2,
    query-strip i1) -> drop / keep / index of a unique [128, 512] 0/1 tile."""
    m2 = np.asarray(mask)[0, 0] != 0  # [S, S], m2[q, k]
    actions = np.full((C2, SP), A_DROP, dtype=np.int64)
    uniq, tiles = {}, []
    for i2 in range(C2):
        for i1 in range(SP):
            blk = m2[i1 * S1B:(i1 + 1) * S1B, i2 * S2B:(i2 + 1) * S2B].T
            if blk.all():
                actions[i2, i1] = A_KEEP
            elif blk.any():
                key = blk.tobytes()
                if key not in uniq:
                    uniq[key] = len(tiles)
                    tiles.append(np.ascontiguousarray(blk).astype(ml_dtypes.bfloat16))
                actions[i2, i1] = uniq[key]
    arr = (np.stack(tiles) if tiles
           else np.zeros((1, S2B, S1B), dtype=ml_dtypes.bfloat16))
    return actions, arr


def _prep(inputs):
    q = np.asarray(inputs["query"], dtype=np.float32).reshape(SEQ, D_MODEL)
    k = np.asarray(inputs["key"], dtype=np.float32).reshape(SEQ, D_MODEL)
    v = np.asarray(inputs["value"], dtype=np.float32).reshape(SEQ, D_MODEL)
    bf = ml_dtypes.bfloat16
    xq = np.ascontiguousarray(q.T).astype(bf)
    xk = np.ascontiguousarray(k.T).astype(bf)
    xv = np.ascontiguousarray(v.T).astype(bf)

    Wq = np.asarray(inputs["Wq"], dtype=np.float32)
    Wk = np.asarray(inputs["Wk"], dtype=np.float32)
    Wv = np.asarray(inputs["Wv"], dtype=np.float32)
    Wo = np.asarray(inputs["Wo"], dtype=np.float32)
    bq = np.asarray(inputs["bq"], dtype=np.float32)
    bk = np.asarray(inputs["bk"], dtype=np.float32)
    bv = np.asarray(inputs["bv"], dtype=np.float32)
    bo = np.asarray(inputs["bo"], dtype=np.float32)

    scale = 1.0 / np.sqrt(D_K)
    actions, mask_tiles = _classify_mask(inputs["mask"])

    # exp-overflow guard for the no-max-subtract softmax (Cauchy-Schwarz bound)
    qn = q @ Wq.T + bq
    kn = k @ Wk.T + bk
    qmax = np.linalg.norm(qn.reshape(SEQ, N_HEADS, D_K), axis=-1).max()
    kmax = np.linalg.norm(kn.reshape(SEQ, N_HEADS, D_K), axis=-1).max()
    assert scale * qmax * kmax < 80.0, "score bound too large for exp without max-subtraction"

    shared = {
        "xq": xq, "xk": xk, "xv": xv,
        "woT": np.ascontiguousarray(Wo.T).astype(bf),
        "bo": np.ascontiguousarray(bo.reshape(KC, 128, 1)),
        "ident": np.eye(128, dtype=np.float32).astype(bf),
        "masks": mask_tiles,
    }
    in_maps = []
    for c in range(N_CORES):
        sl = slice(c * F, (c + 1) * F)
        m = dict(shared)
        m["wq"] = np.ascontiguousarray((Wq[sl] * scale).T).astype(bf)
        m["wk"] = np.ascontiguousarray(Wk[sl].T).astype(bf)
        m["wv"] = np.ascontiguousarray(Wv[sl].T).astype(bf)
        m["bq"] = np.ascontiguousarray((bq[sl] * scale).reshape(F, 1))
        m["bk"] = np.ascontiguousarray(bk[sl].reshape(F, 1))
        m["bv"] = np.ascontiguousarray(bv[sl].reshape(F, 1))
        # indirect-gather row indices into agf flattened as (b, s, g, p):
        # core c owns batch c//2, strips 2*(c%2)+{0,1}; row holds 512 seq.
        bb = c // 2
        rows = np.empty((128, KC, 2), dtype=np.int64)
        pp = np.arange(128)
        for g in range(KC):
            for s in range(2):
                s2 = 2 * (c % 2) + s
                rows[:, g, s] = ((bb * SP + s2) * N_CORES + g) * 128 + pp
        m["oidx"] = np.ascontiguousarray(
            rows.reshape(128, KC * 2).astype(np.int32))
        in_maps.append(m)
    return in_maps, actions, mask_tiles


def _run(inputs, trace=False, trace_cores=None, debug=False):
    in_maps, actions, mask_tiles = _prep(inputs)
    key = (actions.tobytes(), len(mask_tiles), debug)
    if key not in _nc_cache:
        _nc_cache[key] = _build_nc(key[0], key[1], debug=debug)
    nc = _nc_cache[key]
    res = run_bass_kernel_spmd(nc, in_maps, list(range(N_CORES)),
                               trace=trace, trace_cores=trace_cores)
    blk = SEQ // N_CORES
    out = np.empty((SEQ, D_MODEL), dtype=np.float32)
    for c in range(N_CORES):
        out[c * blk:(c + 1) * blk] = res.results[c]["out_t"].T
    return out.reshape(B, S, D_MODEL), res


def kernel(**inputs) -> np.ndarray:
    out, _ = _run(inputs)
    return out


# revision 31
# speedup vs baseline: 1.1241x; 1.0066x over previous
"""Trainium2 Bass kernel for nn_MultiHeadAttention (B=4, S=2048, D=1024, H=16).

Sharding: tensor-parallel over heads (2 heads per core, 8 cores). Each core:
  1. Projects Q/K (feature-major, [128 feats x 8192 seq]) and V (seq-major via
     PE transpose).
  2. Computes causal attention for its 8 (batch, head) pairs in bf16 with
     fp32 PSUM accumulation. PE-array packing: the two heads' score matmuls
     (K=64 contraction) run concurrently in disjoint row-groups; the two
     heads' AV matmuls (M=64) run concurrently in disjoint col-groups; the
     softmax denominators are M=1 matmuls against a ones column, packed into
     col-tiles 0/32. Projection matmuls for batch b+1 are interleaved into
     the attention instruction stream of batch b to keep the PE dense (HAM
     stays un-throttled).
  3. Normalization: reciprocal_approx_fast on the two denominator rows, one
     broadcast matmul expands them across both heads' 64 features, one DVE
     multiply, then a per-strip AllGather (bf16) publishes the [128, 512]
     block. 16 small per-strip gathers replace 4 big per-batch ones so the
     final gather tail is short. Each core then pulls the features of its own
     1024-row sequence block via indirect (index-driven) DMA -- indices come
     from a per-core input tensor so the SPMD program is identical across
     cores -- and computes its block of the output projection. A chained
     dummy-matmul warm keeper spans the final gather wait.
Host wraps: shards weights (with 1/sqrt(dk) folded into Wq), classifies mask
blocks (skip / keep / masked via unique [128, 512] tiles), and reassembles
the full [4, 2048, 1024] output.
"""

import ml_dtypes
import numpy as np

import concourse.bass as bass
import concourse.bacc as bacc
import concourse.mybir as mybir
import concourse.tile as tile
from concourse.bass_utils import run_bass_kernel_spmd

F32 = mybir.dt.float32
F32R = mybir.dt.float32r
BF16 = mybir.dt.bfloat16
AF = mybir.ActivationFunctionType
OP = mybir.AluOpType

B, S, D_MODEL, N_HEADS, D_K = 4, 2048, 1024, 16, 64
N_CORES = 8
HPC = N_HEADS // N_CORES          # heads per core = 2
F = HPC * D_K                     # feature slice per core = 128
SEQ = B * S                       # 8192
S1B = 512                         # query-strip width (scores free dim)
S2B = 128                         # key-block height (scores partition dim)
SP = S // S1B                     # 4 strips per batch
C2 = S // S2B                     # 16 key chunks per batch
KC = D_MODEL // 128               # 8 contraction chunks for projections
A_DROP, A_KEEP = -2, -1

_nc_cache = {}


def _build_nc(actions_key, n_masks, debug=False):
    actions = np.frombuffer(actions_key, dtype=np.int64).reshape(C2, SP)
    nc = bacc.Bacc("TRN2", target_bir_lowering=False, debug=False,
                   num_devices=N_CORES)

    xq = nc.dram_tensor("xq", [D_MODEL, SEQ], BF16, kind="ExternalInput")
    xk = nc.dram_tensor("xk", [D_MODEL, SEQ], BF16, kind="ExternalInput")
    xv = nc.dram_tensor("xv", [D_MODEL, SEQ], BF16, kind="ExternalInput")
    wq = nc.dram_tensor("wq", [D_MODEL, F], BF16, kind="ExternalInput")
    wk = nc.dram_tensor("wk", [D_MODEL, F], BF16, kind="ExternalInput")
    wv = nc.dram_tensor("wv", [D_MODEL, F], BF16, kind="ExternalInput")
    bq = nc.dram_tensor("bq", [F, 1], F32, kind="ExternalInput")
    bk = nc.dram_tensor("bk", [F, 1], F32, kind="ExternalInput")
    bv = nc.dram_tensor("bv", [F, 1], F32, kind="ExternalInput")
    woT = nc.dram_tensor("woT", [D_MODEL, D_MODEL], BF16, kind="ExternalInput")
    bo = nc.dram_tensor("bo", [KC, 128, 1], F32, kind="ExternalInput")
    ident = nc.dram_tensor("ident", [128, 128], BF16, kind="ExternalInput")
    masks = nc.dram_tensor("masks", [max(n_masks, 1), S2B, S1B], BF16,
                           kind="ExternalInput")

    oidx = nc.dram_tensor("oidx", [128, KC * 2], mybir.dt.int32,
                          kind="ExternalInput")
    agin = nc.dram_tensor("agin", [B, SP, F, S1B], BF16)
    agf = nc.dram_tensor("agf", [B, SP, N_CORES, F, S1B], BF16,
                         addr_space="Shared")
    out_t = nc.dram_tensor("out_t", [D_MODEL, SEQ // N_CORES], F32,
                           kind="ExternalOutput")
    if debug:
        dbg_ob = nc.dram_tensor("dbg_ob", [B, SP, F, S1B], BF16,
                                kind="ExternalOutput")
        dbg_avc = nc.dram_tensor("dbg_avc", [B, SP, F, S1B], F32,
                                 kind="ExternalOutput")
        dbg_rcp = nc.dram_tensor("dbg_rcp", [B, SP, 64, S1B], F32,
                                 kind="ExternalOutput")
        dbg_rhs = nc.dram_tensor("dbg_rhs", [128, KC, 2, S1B], BF16,
                                 kind="ExternalOutput")

    with tile.TileContext(nc) as tc:
      with tc.tile_pool(name="oproj_w", bufs=1) as opw:
        wo_sb = opw.tile([128, KC, KC, 128], BF16, tag="wo")
        bo_sb = opw.tile([128, KC], F32, tag="bo")
        with (
            tc.tile_pool(name="const", bufs=1) as cst,
            tc.tile_pool(name="persist", bufs=1) as per,
            tc.tile_pool(name="xin", bufs=16) as xin,
            tc.tile_pool(name="vtmp", bufs=2) as vtmp,
            tc.tile_pool(name="probs", bufs=8) as prp,
            tc.tile_pool(name="norm", bufs=4) as nrm,
            tc.tile_pool(name="obuf", bufs=3) as obp,
            tc.tile_pool(name="pp_ps", bufs=2, space="PSUM") as pp_ps,
            tc.tile_pool(name="sc_ps", bufs=2, space="PSUM") as sc_ps,
            tc.tile_pool(name="av_ps", bufs=2, space="PSUM") as av_ps,
        ):
            wq_sb = cst.tile([128, KC, F], BF16, tag="wq")
            wk_sb = cst.tile([128, KC, F], BF16, tag="wk")
            wv_sb = cst.tile([128, KC, F], BF16, tag="wv")
            nc.sync.dma_start(wq_sb[:], wq[:].rearrange("(kc p) f -> p kc f", p=128))
            nc.sync.dma_start(wk_sb[:], wk[:].rearrange("(kc p) f -> p kc f", p=128))
            nc.sync.dma_start(wv_sb[:], wv[:].rearrange("(kc p) f -> p kc f", p=128))
            bq_sb = cst.tile([F, 1], F32, tag="bq")
            bk_sb = cst.tile([F, 1], F32, tag="bk")
            bv_sb = cst.tile([F, 1], F32, tag="bv")
            nc.sync.dma_start(bq_sb[:], bq[:])
            nc.sync.dma_start(bk_sb[:], bk[:])
            nc.sync.dma_start(bv_sb[:], bv[:])
            id_sb = cst.tile([128, 128], BF16, tag="id")
            nc.sync.dma_start(id_sb[:], ident[:])
            mk_sb = cst.tile([S2B, max(n_masks, 1), S1B], BF16, tag="mk")
            nc.sync.dma_start(mk_sb[:], masks[:].rearrange("n p f -> p n f"))

            qT = per.tile([F, SEQ], BF16, tag="qT")
            kT = per.tile([F, SEQ], BF16, tag="kT")
            # V (seq-major) with per-head ones column for the softmax
            # denominator: [s2_in_chunk, b, c2, h, dk+1]
            v_aug = per.tile([S2B, B, C2, HPC, D_K + 1], BF16, tag="vaug")
            ones_sb = cst.tile([128, 1], F32, tag="ones")
            nc.vector.memset(ones_sb[:], 1.0)
            ones_rb = cst.tile([1, D_K], BF16, tag="onesr")
            nc.vector.memset(ones_rb[:], 1.0)

            # startup warm-up: independent dummy matmuls fill the initial
            # x-DMA wait so the PE reaches K=8/8 before projections start
            warm_rhs = cst.tile([128, S1B], BF16, tag="wrm")
            nc.vector.tensor_copy(
                warm_rhs[:],
                wq_sb[:].rearrange("p kc f -> p (kc f)")[:, 0:S1B])
            for _ in range(16):
                wp0 = pp_ps.tile([128, S1B], F32, tag="pp", name="wp0")
                nc.tensor.matmul(wp0[:], wq_sb[:, 0, :], warm_rhs[:],
                                 start=True, stop=True)

            def make_proj_units(b):
                """Emission closures for batch b's projections; popped as
                PE filler between attention groups of batch b-1."""
                units = []
                xts_map = {}

                def dma_unit():
                    for name, x_dram in (("q", xq), ("k", xk), ("v", xv)):
                        xts = []
                        for kc in range(KC):
                            xt = xin.tile([128, S], BF16, tag="xt")
                            nc.sync.dma_start(
                                xt[:], x_dram[kc * 128:(kc + 1) * 128,
                                              b * S:(b + 1) * S])
                            xts.append(xt)
                        xts_map[name] = xts
                units.append(dma_unit)

                def proj_unit(name, w_sb, b_sb, sc_local):
                    xts = xts_map[name]
                    sl = slice(sc_local * S1B, (sc_local + 1) * S1B)
                    gsl = slice(b * S + sc_local * S1B,
                                b * S + (sc_local + 1) * S1B)
                    ps = pp_ps.tile([128, S1B], F32, tag="pp")
                    for kc in range(KC):
                        nc.tensor.matmul(ps[:], w_sb[:, kc, :], xts[kc][:, sl],
                                         start=(kc == 0), stop=(kc == KC - 1))
                    if name == "q":
                        nc.vector.tensor_scalar_add(qT[:, gsl], ps[:], b_sb[:, 0:1])
                    elif name == "k":
                        nc.vector.tensor_scalar_add(kT[:, gsl], ps[:], b_sb[:, 0:1])
                    else:
                        vt = vtmp.tile([128, S1B], BF16, tag="vt")
                        nc.vector.tensor_scalar_add(vt[:], ps[:], b_sb[:, 0:1])
                        for j in range(S1B // 128):
                            tp = pp_ps.tile([128, 128], BF16, tag="pp")
                            nc.tensor.transpose(tp[:], vt[:, j * 128:(j + 1) * 128],
                                                id_sb[:])
                            c2 = sc_local * (S1B // 128) + j
                            nc.vector.tensor_copy(
                                v_aug[:, b, c2, :, 0:D_K],
                                tp[:].rearrange("p (h d) -> p h d", h=HPC))
                            nc.vector.tensor_copy(
                                v_aug[:, b, c2, :, D_K:D_K + 1],
                                ones_sb[:, :, None].to_broadcast([S2B, HPC, 1]))

                for name, w_sb, b_sb in (("q", wq_sb, bq_sb),
                                         ("k", wk_sb, bk_sb),
                                         ("v", wv_sb, bv_sb)):
                    for sc_local in range(SP):
                        units.append(
                            lambda n=name, w=w_sb, bb=b_sb, s=sc_local:
                                proj_unit(n, w, bb, s))
                return units

            def emit_attention(b, fillers):
                """Attention for batch b; pops filler closures (batch b+1
                projections) between chunk groups to keep the PE dense."""
                # fillers[0] is the DMA unit: emit immediately for max lead.
                fidx = 0
                if fillers:
                    fillers[0]()
                    fidx = 1
                n_groups = sum(
                    len([i2 for i2 in range(C2) if actions[i2, i1] != A_DROP])
                    for i1 in range(SP)) // 2
                gcount = 0
                for i1 in range(SP):
                    kept = [i2 for i2 in range(C2) if actions[i2, i1] != A_DROP]
                    avs = [av_ps.tile([D_K + 1, S1B], F32, tag="av", name="av0"),
                           av_ps.tile([D_K + 1, S1B], F32, tag="av", name="av1")]
                    q_lo = b * S + i1 * S1B

                    def emit_av(pend_g, pend_prs, start_idx, kept=kept,
                                avs=avs):
                        n_k = len(kept)
                        for idx, i2 in enumerate(pend_g):
                            first = (start_idx + idx == 0)
                            last = (start_idx + idx == n_k - 1)
                            for lh in range(HPC):
                                nc.tensor.matmul(
                                    avs[lh][:],
                                    v_aug[:, b, i2, lh, :],
                                    pend_prs[lh][:, idx * S1B:(idx + 1) * S1B],
                                    start=first, stop=last)

                    n_done = 0
                    pend = None
                    while n_done < len(kept):
                        g = kept[n_done:n_done + 2]
                        sc_ts, prs = [], []
                        for lh in range(HPC):
                            sc_ts.append(sc_ps.tile([128, S1B * 2], F32,
                                                    tag="sc", name="sc_t"))
                            prs.append(prp.tile([128, S1B * 2], BF16,
                                                tag="pr", name="pr"))
                        # interleave heads per chunk: disjoint row-groups
                        # (h0: partitions 0-63, h1: 64-127) run concurrently
                        for idx, i2 in enumerate(g):
                            k_lo = b * S + i2 * S2B
                            for lh in range(HPC):
                                r0 = lh * D_K
                                nc.tensor.matmul(
                                    sc_ts[lh][:, idx * S1B:(idx + 1) * S1B],
                                    kT[r0:r0 + D_K, k_lo:k_lo + S2B],
                                    qT[r0:r0 + D_K, q_lo:q_lo + S1B],
                                    start=True, stop=True)
                        for lh in range(HPC):
                            nc.scalar.activation(prs[lh][:, 0:len(g) * S1B],
                                                 sc_ts[lh][:, 0:len(g) * S1B],
                                                 AF.Exp)
                        for lh in range(HPC):
                            for idx, i2 in enumerate(g):
                                a = actions[i2, i1]
                                if a >= 0:
                                    pr_sl = prs[lh][:, idx * S1B:(idx + 1) * S1B]
                                    nc.vector.tensor_tensor(
                                        pr_sl, pr_sl, mk_sb[:, a, :], OP.mult)
                        if pend is not None:
                            emit_av(*pend)
                        pend = (g, prs, n_done)
                        n_done += len(g)
                        gcount += 1
                        while (fidx < len(fillers)
                               and fidx - 1 < (gcount * (len(fillers) - 1)
                                               ) // n_groups):
                            fillers[fidx]()
                            fidx += 1
                    if pend is not None:
                        emit_av(*pend)

                    # normalization per head: copy out of PSUM, fast
                    # reciprocal of the denominator row, broadcast matmul
                    for lh in range(HPC):
                        r0, r1 = lh * D_K, (lh + 1) * D_K
                        avc = nrm.tile([D_K, S1B], F32, tag="avc")
                        nc.vector.tensor_copy(avc[:], avs[lh][0:D_K, :])
                        # shift the denominator row to partition 0 (regular
                        # DVE copy supports base shift; the custom-DVE
                        # reciprocal does not)
                        den = nrm.tile([1, S1B], F32, tag="den")
                        nc.vector.tensor_copy(den[:], avs[lh][D_K:D_K + 1, :])
                        rcp = nrm.tile([1, S1B], F32, tag="rcp")
                        nc.vector.reciprocal_approx_fast(rcp[:], den[:])
                        rcpb = nrm.tile([1, S1B], BF16, tag="rcpb")
                        nc.vector.tensor_copy(rcpb[:], rcp[:])
                        bc_ps = pp_ps.tile([D_K, S1B], F32, tag="pp")
                        nc.tensor.matmul(bc_ps[:], ones_rb[:], rcpb[:],
                                         start=True, stop=True)
                        ob = obp.tile([D_K, S1B], BF16, tag="ob")
                        nc.vector.tensor_tensor(ob[:], avc[:], bc_ps[:],
                                                OP.mult)
                        nc.sync.dma_start(agin[b, i1, r0:r1, :], ob[:])
                        if debug:
                            nc.sync.dma_start(dbg_ob[b, i1, r0:r1, :], ob[:])
                            nc.sync.dma_start(dbg_avc[b, i1, r0:r1, :], avc[:])
                            nc.sync.dma_start(
                                dbg_rcp[b, i1, lh * 32:lh * 32 + 1, :],
                                rcp[:])
                    nc.gpsimd.collective_compute(
                        "AllGather", OP.bypass,
                        ins=[agin[b, i1]], outs=[agf[b, i1]],
                        replica_groups=[list(range(N_CORES))])
                while fidx < len(fillers):
                    fillers[fidx]()
                    fidx += 1

            def make_dummy_units(n):
                """PE keep-warm filler for the last batch (no projections
                left to interleave): independent dummy matmuls."""
                units = [lambda: None]  # slot 0 stands in for the DMA unit
                def dummy():
                    wp = pp_ps.tile([128, S1B], F32, tag="pp", name="wpd")
                    nc.tensor.matmul(wp[:], wq_sb[:, 0, :], warm_rhs[:],
                                     start=True, stop=True)
                    nc.tensor.matmul(wp[:], wk_sb[:, 0, :], warm_rhs[:],
                                     start=True, stop=True)
                for _ in range(n):
                    units.append(dummy)
                return units

            # batch 0 projections up-front, then attention(b) interleaved
            # with projections(b+1)
            units0 = make_proj_units(0)
            for u in units0:
                u()
            for b in range(B):
                fillers = (make_proj_units(b + 1) if b + 1 < B
                           else make_dummy_units(12))
                emit_attention(b, fillers)

        nc.sync.dma_start(
            wo_sb[:],
            woT[:].rearrange("(kc p) (dc f) -> p kc dc f", p=128, f=128))
        nc.sync.dma_start(bo_sb[:], bo[:].rearrange("d p one -> p (d one)"))
        with (
            tc.tile_pool(name="oproj", bufs=1) as opr,
            tc.tile_pool(name="ob_sb", bufs=3) as obp2,
            tc.tile_pool(name="op_ps", bufs=2, space="PSUM") as op_ps,
        ):
            # warm-keeper: independent dummy matmuls span the final gather
            # wait back-to-back so the PE clock stays at 2.4GHz for the
            # output projection
            wsb = opr.tile([128, S1B], BF16, tag="wsb")
            nc.vector.tensor_copy(wsb[:], wo_sb[:, 0, 0:4, :].rearrange("p a f -> p (a f)"))
            for _ in range(56):
                wps = op_ps.tile([128, S1B], F32, tag="op", name="wps")
                nc.tensor.matmul(wps[:], wo_sb[:, 0, 0, :], wsb[:],
                                 start=True, stop=True)
            idx_sb = opr.tile([128, KC * 2], mybir.dt.int32, tag="idx")
            nc.sync.dma_start(idx_sb[:], oidx[:])
            agf_rows = agf[:].rearrange("b s g p f -> (b s g p) f")
            rhs = opr.tile([128, KC, 2, S1B], BF16, tag="rhs")
            for s in range(2):
                for g in range(KC):
                    nc.gpsimd.indirect_dma_start(
                        out=rhs[:, g, s, :], out_offset=None,
                        in_=agf_rows,
                        in_offset=bass.IndirectOffsetOnAxis(
                            ap=idx_sb[:, g * 2 + s:g * 2 + s + 1], axis=0))
            if debug:
                nc.sync.dma_start(dbg_rhs[:], rhs[:])
            for dc in range(KC):
                for sc2 in range(2):
                    ps = op_ps.tile([128, S1B], F32, tag="op")
                    for kc in range(KC):
                        nc.tensor.matmul(
                            ps[:], wo_sb[:, kc, dc, :],
                            rhs[:, kc, sc2, :],
                            start=(kc == 0), stop=(kc == KC - 1))
                    ob = obp2.tile([128, S1B], F32, tag="obt")
                    nc.vector.tensor_scalar_add(ob[:], ps[:], bo_sb[:, dc:dc + 1])
                    nc.sync.dma_start(
                        out_t[dc * 128:(dc + 1) * 128,
                              sc2 * S1B:(sc2 + 1) * S1B], ob[:])

    nc.finalize()
    return nc


def _classify_mask(mask):
    """Block-classify mask[0,0] on the scoresT grid: per (key-chunk i2,
    query-strip i1) -> drop / keep / index of a unique [128, 512] 0/1 tile."""
    m2 = np.asarray(mask)[0, 0] != 0  # [S, S], m2[q, k]
    actions = np.full((C2, SP), A_DROP, dtype=np.int64)
    uniq, tiles = {}, []
    for i2 in range(C2):
        for i1 in range(SP):
            blk = m2[i1 * S1B:(i1 + 1) * S1B, i2 * S2B:(i2 + 1) * S2B].T
            if blk.all():
                actions[i2, i1] = A_KEEP
            elif blk.any():
                key = blk.tobytes()
                if key not in uniq:
                    uniq[key] = len(tiles)
                    tiles.append(np.ascontiguousarray(blk).astype(ml_dtypes.bfloat16))
                actions[i2, i1] = uniq[key]
    arr = (np.stack(tiles) if tiles
           else np.zeros((1, S2B, S1B), dtype=ml_dtypes.bfloat16))
    return actions, arr


def _prep(inputs):
    q = np.asarray(inputs["query"], dtype=np.float32).reshape(SEQ, D_MODEL)
    k = np.asarray(inputs["key"], dtype=np.float32).reshape(SEQ, D_MODEL)
    v = np.asarray(inputs["value"], dtype=np.float32).reshape(SEQ, D_MODEL)
    bf = ml_dtypes.bfloat16
    xq = np.ascontiguousarray(q.T).astype(bf)
    xk = np.ascontiguousarray(k.T).astype(bf)
    xv = np.ascontiguousarray(v.T).astype(bf)

    Wq = np.asarray(inputs["Wq"], dtype=np.float32)
    Wk = np.asarray(inputs["Wk"], dtype=np.float32)
    Wv = np.asarray(inputs["Wv"], dtype=np.float32)
    Wo = np.asarray(inputs["Wo"], dtype=np.float32)
    bq = np.asarray(inputs["bq"], dtype=np.float32)
    bk = np.asarray(inputs["bk"], dtype=np.float32)
    bv = np.asarray(inputs["bv"], dtype=np.float32)
    bo = np.asarray(inputs["bo"], dtype=np.float32)

    scale = 1.0 / np.sqrt(D_K)
    actions, mask_tiles = _classify_mask(inputs["mask"])

    # exp-overflow guard for the no-max-subtract softmax (Cauchy-Schwarz bound)
    qn = q @ Wq.T + bq
    kn = k @ Wk.T + bk
    qmax = np.linalg.norm(qn.reshape(SEQ, N_HEADS, D_K), axis=-1).max()
    kmax = np.linalg.norm(kn.reshape(SEQ, N_HEADS, D_K), axis=-1).max()
    assert scale * qmax * kmax < 80.0, "score bound too large for exp without max-subtraction"

    shared = {
        "xq": xq, "xk": xk, "xv": xv,
        "woT": np.ascontiguousarray(Wo.T).astype(bf),
        "bo": np.ascontiguousarray(bo.reshape(KC, 128, 1)),
        "ident": np.eye(128, dtype=np.float32).astype(bf),
        "masks": mask_tiles,
    }
    in_maps = []
    for c in range(N_CORES):
        sl = slice(c * F, (c + 1) * F)
        m = dict(shared)
        m["wq"] = np.ascontiguousarray((Wq[sl] * scale).T).astype(bf)
        m["wk"] = np.ascontiguousarray(Wk[sl].T).astype(bf)
        m["wv"] = np.ascontiguousarray(Wv[sl].T).astype(bf)
        m["bq"] = np.ascontiguousarray((bq[sl] * scale).reshape(F, 1))
        m["bk"] = np.ascontiguousarray(bk[sl].reshape(F, 1))
        m["bv"] = np.ascontiguousarray(bv[sl].reshape(F, 1))
        # indirect-gather row indices into agf flattened as (b, s, g, p):
        # core c owns batch c//2, strips 2*(c%2)+{0,1}; row holds 512 seq.
        bb = c // 2
        rows = np.empty((128, KC, 2), dtype=np.int64)
        pp = np.arange(128)
        for g in range(KC):
            for s in range(2):
                s2 = 2 * (c % 2) + s
                rows[:, g, s] = ((bb * SP + s2) * N_CORES + g) * 128 + pp
        m["oidx"] = np.ascontiguousarray(
            rows.reshape(128, KC * 2).astype(np.int32))
        in_maps.append(m)
    return in_maps, actions, mask_tiles


def _run(inputs, trace=False, trace_cores=None, debug=False):
    in_maps, actions, mask_tiles = _prep(inputs)
    key = (actions.tobytes(), len(mask_tiles), debug)
    if key not in _nc_cache:
        _nc_cache[key] = _build_nc(key[0], key[1], debug=debug)
    nc = _nc_cache[key]
    res = run_bass_kernel_spmd(nc, in_maps, list(range(N_CORES)),
                               trace=trace, trace_cores=trace_cores)
    blk = SEQ // N_CORES
    out = np.empty((SEQ, D_MODEL), dtype=np.float32)
    for c in range(N_CORES):
        out[c * blk:(c + 1) * blk] = res.results[c]["out_t"].T
    return out.reshape(B, S, D_MODEL), res


def kernel(**inputs) -> np.ndarray:
    out, _ = _run(inputs)
    return out


# revision 37
# speedup vs baseline: 1.1413x; 1.0153x over previous
"""Trainium2 Bass kernel for nn_MultiHeadAttention (B=4, S=2048, D=1024, H=16).

Sharding: tensor-parallel over heads (2 heads per core, 8 cores). Each core:
  1. Projects Q/K (feature-major, [128 feats x 8192 seq]) and V (seq-major via
     PE transpose).
  2. Computes causal attention for its 8 (batch, head) pairs in bf16 with
     fp32 PSUM accumulation. PE-array packing: the two heads' score matmuls
     (K=64 contraction) run concurrently in disjoint row-groups; the two
     heads' AV matmuls (M=64) run concurrently in disjoint col-groups; the
     softmax denominators are M=1 matmuls against a ones column, packed into
     col-tiles 0/32. Projection matmuls for batch b+1 are interleaved into
     the attention instruction stream of batch b to keep the PE dense (HAM
     stays un-throttled).
  3. Normalization: reciprocal_approx_fast on the two denominator rows, one
     broadcast matmul expands them across both heads' 64 features, one DVE
     multiply, then a per-strip AllGather (bf16) publishes the [128, 512]
     block. 16 small per-strip gathers replace 4 big per-batch ones so the
     final gather tail is short. Each core then pulls the features of its own
     1024-row sequence block via indirect (index-driven) DMA -- indices come
     from a per-core input tensor so the SPMD program is identical across
     cores -- and computes its block of the output projection. A chained
     dummy-matmul warm keeper spans the final gather wait.
Host wraps: shards weights (with 1/sqrt(dk) folded into Wq), classifies mask
blocks (skip / keep / masked via unique [128, 512] tiles), and reassembles
the full [4, 2048, 1024] output.
"""

import ml_dtypes
import numpy as np

import concourse.bass as bass
import concourse.bacc as bacc
import concourse.mybir as mybir
import concourse.tile as tile
from concourse.bass_utils import run_bass_kernel_spmd

F32 = mybir.dt.float32
F32R = mybir.dt.float32r
BF16 = mybir.dt.bfloat16
AF = mybir.ActivationFunctionType
OP = mybir.AluOpType

B, S, D_MODEL, N_HEADS, D_K = 4, 2048, 1024, 16, 64
N_CORES = 8
HPC = N_HEADS // N_CORES          # heads per core = 2
F = HPC * D_K                     # feature slice per core = 128
SEQ = B * S                       # 8192
S1B = 512                         # query-strip width (scores free dim)
S2B = 128                         # key-block height (scores partition dim)
SP = S // S1B                     # 4 strips per batch
C2 = S // S2B                     # 16 key chunks per batch
KC = D_MODEL // 128               # 8 contraction chunks for projections
A_DROP, A_KEEP = -2, -1

_nc_cache = {}


def _build_nc(actions_key, n_masks, debug=False):
    actions = np.frombuffer(actions_key, dtype=np.int64).reshape(C2, SP)
    nc = bacc.Bacc("TRN2", target_bir_lowering=False, debug=False,
                   num_devices=N_CORES)

    xq = nc.dram_tensor("xq", [D_MODEL, SEQ], BF16, kind="ExternalInput")
    xk = nc.dram_tensor("xk", [D_MODEL, SEQ], BF16, kind="ExternalInput")
    xv = nc.dram_tensor("xv", [D_MODEL, SEQ], BF16, kind="ExternalInput")
    wq = nc.dram_tensor("wq", [D_MODEL, F], BF16, kind="ExternalInput")
    wk = nc.dram_tensor("wk", [D_MODEL, F], BF16, kind="ExternalInput")
    wv = nc.dram_tensor("wv", [D_MODEL, F], BF16, kind="ExternalInput")
    bq = nc.dram_tensor("bq", [F, 1], F32, kind="ExternalInput")
    bk = nc.dram_tensor("bk", [F, 1], F32, kind="ExternalInput")
    bv = nc.dram_tensor("bv", [F, 1], F32, kind="ExternalInput")
    woT = nc.dram_tensor("woT", [D_MODEL, D_MODEL], BF16, kind="ExternalInput")
    bo = nc.dram_tensor("bo", [KC, 128, 1], F32, kind="ExternalInput")
    ident = nc.dram_tensor("ident", [128, 128], BF16, kind="ExternalInput")
    masks = nc.dram_tensor("masks", [max(n_masks, 1), S2B, S1B], BF16,
                           kind="ExternalInput")

    oidx = nc.dram_tensor("oidx", [128, KC], mybir.dt.int32,
                          kind="ExternalInput")
    agin = nc.dram_tensor("agin", [B, 2, F, 2 * S1B], BF16)
    agf = nc.dram_tensor("agf", [B, 2, N_CORES, F, 2 * S1B], BF16,
                         addr_space="Shared")
    out_t = nc.dram_tensor("out_t", [D_MODEL, SEQ // N_CORES], F32,
                           kind="ExternalOutput")
    if debug:
        dbg_ob = nc.dram_tensor("dbg_ob", [B, SP, F, S1B], BF16,
                                kind="ExternalOutput")
        dbg_avc = nc.dram_tensor("dbg_avc", [B, SP, F, S1B], F32,
                                 kind="ExternalOutput")
        dbg_rcp = nc.dram_tensor("dbg_rcp", [B, SP, 64, S1B], F32,
                                 kind="ExternalOutput")
        dbg_rhs = nc.dram_tensor("dbg_rhs", [128, KC, 2 * S1B], BF16,
                                 kind="ExternalOutput")

    with tile.TileContext(nc) as tc:
      with tc.tile_pool(name="oproj_w", bufs=1) as opw:
        wo_sb = opw.tile([128, KC, KC, 128], BF16, tag="wo")
        bo_sb = opw.tile([128, KC], F32, tag="bo")
        with (
            tc.tile_pool(name="const", bufs=1) as cst,
            tc.tile_pool(name="persist", bufs=1) as per,
            tc.tile_pool(name="xin", bufs=16) as xin,
            tc.tile_pool(name="vtmp", bufs=2) as vtmp,
            tc.tile_pool(name="probs", bufs=8) as prp,
            tc.tile_pool(name="norm", bufs=4) as nrm,
            tc.tile_pool(name="obuf", bufs=3) as obp,
            tc.tile_pool(name="pp_ps", bufs=2, space="PSUM") as pp_ps,
            tc.tile_pool(name="sc_ps", bufs=2, space="PSUM") as sc_ps,
            tc.tile_pool(name="av_ps", bufs=2, space="PSUM") as av_ps,
        ):
            wq_sb = cst.tile([128, KC, F], BF16, tag="wq")
            wk_sb = cst.tile([128, KC, F], BF16, tag="wk")
            wv_sb = cst.tile([128, KC, F], BF16, tag="wv")
            nc.sync.dma_start(wq_sb[:], wq[:].rearrange("(kc p) f -> p kc f", p=128))
            nc.sync.dma_start(wk_sb[:], wk[:].rearrange("(kc p) f -> p kc f", p=128))
            nc.sync.dma_start(wv_sb[:], wv[:].rearrange("(kc p) f -> p kc f", p=128))
            bq_sb = cst.tile([F, 1], F32, tag="bq")
            bk_sb = cst.tile([F, 1], F32, tag="bk")
            bv_sb = cst.tile([F, 1], F32, tag="bv")
            nc.sync.dma_start(bq_sb[:], bq[:])
            nc.sync.dma_start(bk_sb[:], bk[:])
            nc.sync.dma_start(bv_sb[:], bv[:])
            id_sb = cst.tile([128, 128], BF16, tag="id")
            nc.sync.dma_start(id_sb[:], ident[:])
            mk_sb = cst.tile([S2B, max(n_masks, 1), S1B], BF16, tag="mk")
            nc.sync.dma_start(mk_sb[:], masks[:].rearrange("n p f -> p n f"))

            qT = per.tile([F, SEQ], BF16, tag="qT")
            kT = per.tile([F, SEQ], BF16, tag="kT")
            # V (seq-major) with per-head ones column for the softmax
            # denominator: [s2_in_chunk, b, c2, h, dk+1]
            v_aug = per.tile([S2B, B, C2, HPC, D_K + 1], BF16, tag="vaug")
            ones_sb = cst.tile([128, 1], F32, tag="ones")
            nc.vector.memset(ones_sb[:], 1.0)
            ones_rb = cst.tile([1, D_K], BF16, tag="onesr")
            nc.vector.memset(ones_rb[:], 1.0)

            # startup warm-up: independent dummy matmuls fill the initial
            # x-DMA wait so the PE reaches K=8/8 before projections start
            warm_rhs = cst.tile([128, S1B], BF16, tag="wrm")
            nc.vector.tensor_copy(
                warm_rhs[:],
                wq_sb[:].rearrange("p kc f -> p (kc f)")[:, 0:S1B])
            for _ in range(16):
                wp0 = pp_ps.tile([128, S1B], F32, tag="pp", name="wp0")
                nc.tensor.matmul(wp0[:], wq_sb[:, 0, :], warm_rhs[:],
                                 start=True, stop=True)

            def make_proj_units(b):
                """Emission closures for batch b's projections; popped as
                PE filler between attention groups of batch b-1."""
                units = []
                xts_map = {}

                def dma_unit():
                    for name, x_dram in (("q", xq), ("k", xk), ("v", xv)):
                        xts = []
                        for kc in range(KC):
                            xt = xin.tile([128, S], BF16, tag="xt")
                            nc.sync.dma_start(
                                xt[:], x_dram[kc * 128:(kc + 1) * 128,
                                              b * S:(b + 1) * S])
                            xts.append(xt)
                        xts_map[name] = xts
                units.append(dma_unit)

                def proj_unit(name, w_sb, b_sb, sc_local):
                    xts = xts_map[name]
                    sl = slice(sc_local * S1B, (sc_local + 1) * S1B)
                    gsl = slice(b * S + sc_local * S1B,
                                b * S + (sc_local + 1) * S1B)
                    ps = pp_ps.tile([128, S1B], F32, tag="pp")
                    for kc in range(KC):
                        nc.tensor.matmul(ps[:], w_sb[:, kc, :], xts[kc][:, sl],
                                         start=(kc == 0), stop=(kc == KC - 1))
                    if name == "q":
                        nc.vector.tensor_scalar_add(qT[:, gsl], ps[:], b_sb[:, 0:1])
                    elif name == "k":
                        nc.vector.tensor_scalar_add(kT[:, gsl], ps[:], b_sb[:, 0:1])
                    else:
                        vt = vtmp.tile([128, S1B], BF16, tag="vt")
                        nc.vector.tensor_scalar_add(vt[:], ps[:], b_sb[:, 0:1])
                        for j in range(S1B // 128):
                            tp = pp_ps.tile([128, 128], BF16, tag="pp")
                            nc.tensor.transpose(tp[:], vt[:, j * 128:(j + 1) * 128],
                                                id_sb[:])
                            c2 = sc_local * (S1B // 128) + j
                            nc.vector.tensor_copy(
                                v_aug[:, b, c2, :, 0:D_K],
                                tp[:].rearrange("p (h d) -> p h d", h=HPC))
                            nc.vector.tensor_copy(
                                v_aug[:, b, c2, :, D_K:D_K + 1],
                                ones_sb[:, :, None].to_broadcast([S2B, HPC, 1]))

                for name, w_sb, b_sb in (("q", wq_sb, bq_sb),
                                         ("k", wk_sb, bk_sb),
                                         ("v", wv_sb, bv_sb)):
                    for sc_local in range(SP):
                        units.append(
                            lambda n=name, w=w_sb, bb=b_sb, s=sc_local:
                                proj_unit(n, w, bb, s))
                return units

            def emit_attention(b, fillers):
                """Attention for batch b; pops filler closures (batch b+1
                projections) between chunk groups to keep the PE dense."""
                # fillers[0] is the DMA unit: emit immediately for max lead.
                fidx = 0
                if fillers:
                    fillers[0]()
                    fidx = 1
                n_groups = sum(
                    len([i2 for i2 in range(C2) if actions[i2, i1] != A_DROP])
                    for i1 in range(SP)) // 2
                gcount = 0
                for i1 in range(SP):
                    kept = [i2 for i2 in range(C2) if actions[i2, i1] != A_DROP]
                    avs = [av_ps.tile([D_K + 1, S1B], F32, tag="av", name="av0"),
                           av_ps.tile([D_K + 1, S1B], F32, tag="av", name="av1")]
                    q_lo = b * S + i1 * S1B

                    def emit_av(pend_g, pend_prs, start_idx, kept=kept,
                                avs=avs):
                        n_k = len(kept)
                        for idx, i2 in enumerate(pend_g):
                            first = (start_idx + idx == 0)
                            last = (start_idx + idx == n_k - 1)
                            for lh in range(HPC):
                                nc.tensor.matmul(
                                    avs[lh][:],
                                    v_aug[:, b, i2, lh, :],
                                    pend_prs[lh][:, idx * S1B:(idx + 1) * S1B],
                                    start=first, stop=last)

                    n_done = 0
                    pend = None
                    while n_done < len(kept):
                        g = kept[n_done:n_done + 2]
                        sc_ts, prs = [], []
                        for lh in range(HPC):
                            sc_ts.append(sc_ps.tile([128, S1B * 2], F32,
                                                    tag="sc", name="sc_t"))
                            prs.append(prp.tile([128, S1B * 2], BF16,
                                                tag="pr", name="pr"))
                        # interleave heads per chunk: disjoint row-groups
                        # (h0: partitions 0-63, h1: 64-127) run concurrently
                        for idx, i2 in enumerate(g):
                            k_lo = b * S + i2 * S2B
                            for lh in range(HPC):
                                r0 = lh * D_K
                                nc.tensor.matmul(
                                    sc_ts[lh][:, idx * S1B:(idx + 1) * S1B],
                                    kT[r0:r0 + D_K, k_lo:k_lo + S2B],
                                    qT[r0:r0 + D_K, q_lo:q_lo + S1B],
                                    start=True, stop=True)
                        for lh in range(HPC):
                            nc.scalar.activation(prs[lh][:, 0:len(g) * S1B],
                                                 sc_ts[lh][:, 0:len(g) * S1B],
                                                 AF.Exp)
                        for lh in range(HPC):
                            for idx, i2 in enumerate(g):
                                a = actions[i2, i1]
                                if a >= 0:
                                    pr_sl = prs[lh][:, idx * S1B:(idx + 1) * S1B]
                                    nc.vector.tensor_tensor(
                                        pr_sl, pr_sl, mk_sb[:, a, :], OP.mult)
                        if pend is not None:
                            emit_av(*pend)
                        pend = (g, prs, n_done)
                        n_done += len(g)
                        gcount += 1
                        while (fidx < len(fillers)
                               and fidx - 1 < (gcount * (len(fillers) - 1)
                                               ) // n_groups):
                            fillers[fidx]()
                            fidx += 1
                    if pend is not None:
                        emit_av(*pend)

                    # normalization per head: copy out of PSUM, fast
                    # reciprocal of the denominator row, broadcast matmul
                    for lh in range(HPC):
                        r0, r1 = lh * D_K, (lh + 1) * D_K
                        avc = nrm.tile([D_K, S1B], F32, tag="avc")
                        nc.vector.tensor_copy(avc[:], avs[lh][0:D_K, :])
                        # shift the denominator row to partition 0 (regular
                        # DVE copy supports base shift; the custom-DVE
                        # reciprocal does not)
                        den = nrm.tile([1, S1B], F32, tag="den")
                        nc.vector.tensor_copy(den[:], avs[lh][D_K:D_K + 1, :])
                        rcp = nrm.tile([1, S1B], F32, tag="rcp")
                        nc.vector.reciprocal_approx_fast(rcp[:], den[:])
                        rcpb = nrm.tile([1, S1B], BF16, tag="rcpb")
                        nc.vector.tensor_copy(rcpb[:], rcp[:])
                        bc_ps = pp_ps.tile([D_K, S1B], F32, tag="pp")
                        nc.tensor.matmul(bc_ps[:], ones_rb[:], rcpb[:],
                                         start=True, stop=True)
                        ob = obp.tile([D_K, S1B], BF16, tag="ob")
                        nc.vector.tensor_tensor(ob[:], avc[:], bc_ps[:],
                                                OP.mult)
                        nc.sync.dma_start(
                            agin[b, i1 // 2, r0:r1,
                                 (i1 % 2) * S1B:(i1 % 2 + 1) * S1B], ob[:])
                        if debug:
                            nc.sync.dma_start(dbg_ob[b, i1, r0:r1, :], ob[:])
                            nc.sync.dma_start(dbg_avc[b, i1, r0:r1, :], avc[:])
                            nc.sync.dma_start(
                                dbg_rcp[b, i1, lh * 32:lh * 32 + 1, :],
                                rcp[:])
                    if i1 % 2 == 1:
                        nc.gpsimd.collective_compute(
                            "AllGather", OP.bypass,
                            ins=[agin[b, i1 // 2]], outs=[agf[b, i1 // 2]],
                            replica_groups=[list(range(N_CORES))])
                while fidx < len(fillers):
                    fillers[fidx]()
                    fidx += 1

            def make_dummy_units(n):
                """PE keep-warm filler for the last batch (no projections
                left to interleave): independent dummy matmuls."""
                units = [lambda: None]  # slot 0 stands in for the DMA unit
                def dummy():
                    wp = pp_ps.tile([128, S1B], F32, tag="pp", name="wpd")
                    nc.tensor.matmul(wp[:], wq_sb[:, 0, :], warm_rhs[:],
                                     start=True, stop=True)
                    nc.tensor.matmul(wp[:], wk_sb[:, 0, :], warm_rhs[:],
                                     start=True, stop=True)
                for _ in range(n):
                    units.append(dummy)
                return units

            # batch 0 projections up-front, then attention(b) interleaved
            # with projections(b+1)
            units0 = make_proj_units(0)
            for u in units0:
                u()
            for b in range(B):
                fillers = (make_proj_units(b + 1) if b + 1 < B
                           else make_dummy_units(12))
                emit_attention(b, fillers)

        nc.sync.dma_start(
            wo_sb[:],
            woT[:].rearrange("(kc p) (dc f) -> p kc dc f", p=128, f=128))
        nc.sync.dma_start(bo_sb[:], bo[:].rearrange("d p one -> p (d one)"))
        with (
            tc.tile_pool(name="oproj", bufs=1) as opr,
            tc.tile_pool(name="ob_sb", bufs=3) as obp2,
            tc.tile_pool(name="op_ps", bufs=2, space="PSUM") as op_ps,
        ):
            # warm-keeper: chained dummy matmuls (paced by DVE copies) span
            # the final gather wait so the PE clock stays at 2.4GHz for the
            # output projection
            wsb = opr.tile([128, S1B], BF16, tag="wsb")
            nc.vector.tensor_copy(wsb[:], wo_sb[:, 0, 0:4, :].rearrange("p a f -> p (a f)"))
            for _ in range(56):
                wps = op_ps.tile([128, S1B], F32, tag="op", name="wps")
                nc.tensor.matmul(wps[:], wo_sb[:, 0, 0, :], wsb[:],
                                 start=True, stop=True)
                wsb = opr.tile([128, S1B], BF16, tag="wsb")
                nc.vector.tensor_copy(wsb[:], wps[:])
            idx_sb = opr.tile([128, KC], mybir.dt.int32, tag="idx")
            nc.sync.dma_start(idx_sb[:], oidx[:])
            agf_rows = agf[:].rearrange("b s g p f -> (b s g p) f")
            rhs = opr.tile([128, KC, 2 * S1B], BF16, tag="rhs")
            for g in range(KC):
                nc.gpsimd.indirect_dma_start(
                    out=rhs[:, g, :], out_offset=None,
                    in_=agf_rows,
                    in_offset=bass.IndirectOffsetOnAxis(
                        ap=idx_sb[:, g:g + 1], axis=0))
            if debug:
                nc.sync.dma_start(dbg_rhs[:], rhs[:])
            for dc in range(KC):
                for sc2 in range(2):
                    ps = op_ps.tile([128, S1B], F32, tag="op")
                    for kc in range(KC):
                        nc.tensor.matmul(
                            ps[:], wo_sb[:, kc, dc, :],
                            rhs[:, kc, sc2 * S1B:(sc2 + 1) * S1B],
                            start=(kc == 0), stop=(kc == KC - 1))
                    ob = obp2.tile([128, S1B], F32, tag="obt")
                    nc.vector.tensor_scalar_add(ob[:], ps[:], bo_sb[:, dc:dc + 1])
                    nc.sync.dma_start(
                        out_t[dc * 128:(dc + 1) * 128,
                              sc2 * S1B:(sc2 + 1) * S1B], ob[:])

    nc.finalize()
    return nc


def _classify_mask(mask):
    """Block-classify mask[0,0] on the scoresT grid: per (key-chunk i2,
    query-strip i1) -> drop / keep / index of a unique [128, 512] 0/1 tile."""
    m2 = np.asarray(mask)[0, 0] != 0  # [S, S], m2[q, k]
    actions = np.full((C2, SP), A_DROP, dtype=np.int64)
    uniq, tiles = {}, []
    for i2 in range(C2):
        for i1 in range(SP):
            blk = m2[i1 * S1B:(i1 + 1) * S1B, i2 * S2B:(i2 + 1) * S2B].T
            if blk.all():
                actions[i2, i1] = A_KEEP
            elif blk.any():
                key = blk.tobytes()
                if key not in uniq:
                    uniq[key] = len(tiles)
                    tiles.append(np.ascontiguousarray(blk).astype(ml_dtypes.bfloat16))
                actions[i2, i1] = uniq[key]
    arr = (np.stack(tiles) if tiles
           else np.zeros((1, S2B, S1B), dtype=ml_dtypes.bfloat16))
    return actions, arr


def _prep(inputs):
    q = np.asarray(inputs["query"], dtype=np.float32).reshape(SEQ, D_MODEL)
    k = np.asarray(inputs["key"], dtype=np.float32).reshape(SEQ, D_MODEL)
    v = np.asarray(inputs["value"], dtype=np.float32).reshape(SEQ, D_MODEL)
    bf = ml_dtypes.bfloat16
    xq = np.ascontiguousarray(q.T).astype(bf)
    xk = np.ascontiguousarray(k.T).astype(bf)
    xv = np.ascontiguousarray(v.T).astype(bf)

    Wq = np.asarray(inputs["Wq"], dtype=np.float32)
    Wk = np.asarray(inputs["Wk"], dtype=np.float32)
    Wv = np.asarray(inputs["Wv"], dtype=np.float32)
    Wo = np.asarray(inputs["Wo"], dtype=np.float32)
    bq = np.asarray(inputs["bq"], dtype=np.float32)
    bk = np.asarray(inputs["bk"], dtype=np.float32)
    bv = np.asarray(inputs["bv"], dtype=np.float32)
    bo = np.asarray(inputs["bo"], dtype=np.float32)

    scale = 1.0 / np.sqrt(D_K)
    actions, mask_tiles = _classify_mask(inputs["mask"])

    # exp-overflow guard for the no-max-subtract softmax (Cauchy-Schwarz bound)
    qn = q @ Wq.T + bq
    kn = k @ Wk.T + bk
    qmax = np.linalg.norm(qn.reshape(SEQ, N_HEADS, D_K), axis=-1).max()
    kmax = np.linalg.norm(kn.reshape(SEQ, N_HEADS, D_K), axis=-1).max()
    assert scale * qmax * kmax < 80.0, "score bound too large for exp without max-subtraction"

    shared = {
        "xq": xq, "xk": xk, "xv": xv,
        "woT": np.ascontiguousarray(Wo.T).astype(bf),
        "bo": np.ascontiguousarray(bo.reshape(KC, 128, 1)),
        "ident": np.eye(128, dtype=np.float32).astype(bf),
        "masks": mask_tiles,
    }
    in_maps = []
    for c in range(N_CORES):
        sl = slice(c * F, (c + 1) * F)
        m = dict(shared)
        m["wq"] = np.ascontiguousarray((Wq[sl] * scale).T).astype(bf)
        m["wk"] = np.ascontiguousarray(Wk[sl].T).astype(bf)
        m["wv"] = np.ascontiguousarray(Wv[sl].T).astype(bf)
        m["bq"] = np.ascontiguousarray((bq[sl] * scale).reshape(F, 1))
        m["bk"] = np.ascontiguousarray(bk[sl].reshape(F, 1))
        m["bv"] = np.ascontiguousarray(bv[sl].reshape(F, 1))
        # indirect-gather row indices into agf flattened as (b, sp, g, p):
        # core c owns batch c//2, strip-pair c%2; each row holds 1024 seq.
        bb, sp = c // 2, c % 2
        rows = np.empty((128, KC), dtype=np.int64)
        pp = np.arange(128)
        for g in range(KC):
            rows[:, g] = ((bb * 2 + sp) * N_CORES + g) * 128 + pp
        m["oidx"] = np.ascontiguousarray(rows.astype(np.int32))
        in_maps.append(m)
    return in_maps, actions, mask_tiles


def _run(inputs, trace=False, trace_cores=None, debug=False):
    in_maps, actions, mask_tiles = _prep(inputs)
    key = (actions.tobytes(), len(mask_tiles), debug)
    if key not in _nc_cache:
        _nc_cache[key] = _build_nc(key[0], key[1], debug=debug)
    nc = _nc_cache[key]
    res = run_bass_kernel_spmd(nc, in_maps, list(range(N_CORES)),
                               trace=trace, trace_cores=trace_cores)
    blk = SEQ // N_CORES
    out = np.empty((SEQ, D_MODEL), dtype=np.float32)
    for c in range(N_CORES):
        out[c * blk:(c + 1) * blk] = res.results[c]["out_t"].T
    return out.reshape(B, S, D_MODEL), res


def kernel(**inputs) -> np.ndarray:
    out, _ = _run(inputs)
    return out
